# revision 46
# baseline (speedup 1.0000x reference)
"""Multi-head cross-attention on Trainium2, 8-core SPMD (bf16 matmul path).

Problem (hardcoded): B=4, T=2048, D=1024, H=16 heads, head_dim=64.
    kv = x_enc @ Wkv + bkv ; q = x_dec @ Wq + bq
    per head: S = q_h k_h^T / sqrt(64); P = softmax(S + mask); O_h = P v_h
    out = concat_h(O_h) @ Wo + bo

Sharding: data parallel over batch (4 slices x 2 cores each) and tensor
parallel over heads within each pair (8 heads per core).  Each core
computes a partial output  Y_local @ Wo[rows_local]  (+bo on the even
core of the pair); the host unshards by summing each pair's partials
and stacking the 4 batch slices.  Host-side shard prep pre-transposes
the activation matrices, regroups Wkv columns, and converts all matmul
operands to bf16 (fp32 accumulation in PSUM; rel-err budget 2e-2).

The mask input is structurally zero for this problem; softmax(S+0) ==
softmax(S), so the kernel does not load it (checked on the host).

Per-core pipeline (bf16 matmul inputs, fp32 PSUM):
  A+B (merged, per 512-token chunk): K^T = Wkv_K^T x_enc^T (head-pair-
     stacked); V natural directly via x-chunk-stationary matmuls
     (out[tok, vch] = x_sub^T Wv) with the V bias added on the
     PSUM->SBUF copy and a per-head ones column for the softmax
     denominator; Q^T = Wq^T x_dec^T into two parity copies whose
     opposite-parity partitions are zeroed once at t=0.  x tiles stream
     in [128, 1024] double-chunks on the gpsimd DMA queue, weights on
     the sync queue, tiny bias loads on the scalar queue (DMA
     descriptor issue is serial, ~0.6us each, so queue placement and
     transfer sizing dominate the phase ramp).
  C: one software-pipelined stream over all (q-chunk, head-pair) items
     x 16 k-tiles: per step, score matmuls S^T = (K^T kt)^T Q^T_parity
     at n, exp(S^T/8) on ACT (bf16 out) at n-1, O'^T = V_aug^T P^T
     PSUM-accumulation at n-3.  Item boundaries overlap inside the
     stream so neither PE nor ACT ever drains (phase C runs at the ACT
     exp roofline).  Normalization is spread over 4 trailing steps
     (PSUM row copy -> PE ones-matmul denominator broadcast -> fast
     reciprocal -> multiplies) and the out-projection is issued in
     per-128-row halves at fixed steps one q-chunk behind.
  D drain: the last q-chunk's out-projection runs after the stream with
     the freed PSUM banks, bias folded in via a ones-row matmul and the
     PSUM->SBUF copy on the then-idle ACT engine; bf16 stores.
"""

import ml_dtypes
import numpy as np

import concourse.bass as bass
import concourse.mybir as mybir
import concourse.tile as tile
from concourse import bacc
from concourse.bass_utils import run_bass_kernel_spmd

f32 = mybir.dt.float32
bf16 = mybir.dt.bfloat16
AF = mybir.ActivationFunctionType
ALU = mybir.AluOpType

P = 128
NP_BF16 = ml_dtypes.bfloat16


def build_nc(T=2048, D=1024, HPC=8, HD=64, n_cores=8):
    """Build + compile the per-core Bass program. HPC = heads per core."""
    assert HD == 64 and HPC % 2 == 0 and T % 512 == 0 and D % P == 0
    CPC = HPC * HD          # q/out channels per core (512)
    TC = 512                # token chunk (psum free dim), phases A/B
    QC = 512                # q chunk, attention phase
    NQ = T // TC            # token chunks (4)
    ND = D // P             # model-dim chunks (8)
    NG = HPC // 2           # head pairs (4)
    NKT = T // P            # k-token tiles (16)
    NSUB = TC // P          # 128-token subtiles per chunk (4)
    HD1 = HD + 1            # V columns + ones column
    SCALE = float(1.0 / np.sqrt(HD))
    ON = min(512, D)        # out-proj free chunk
    NON = D // ON
    NQC = T // QC

    nc = bacc.Bacc("TRN2", target_bir_lowering=False, debug=False,
                   enable_asserts=False, num_devices=n_cores)

    xeT = nc.dram_tensor("x_enc_t", [D, T], bf16, kind="ExternalInput").ap()
    xdT = nc.dram_tensor("x_dec_t", [D, T], bf16, kind="ExternalInput").ap()
    wq_d = nc.dram_tensor("wq", [D, CPC], bf16, kind="ExternalInput").ap()
    wkv_d = nc.dram_tensor("wkv_g", [D, 2 * CPC], bf16, kind="ExternalInput").ap()
    wo_d = nc.dram_tensor("wo", [CPC, D], bf16, kind="ExternalInput").ap()
    bq_d = nc.dram_tensor("bq", [CPC], f32, kind="ExternalInput").ap()
    bkv_d = nc.dram_tensor("bkv_g", [2 * CPC], f32, kind="ExternalInput").ap()
    bo_d = nc.dram_tensor("bo", [D], f32, kind="ExternalInput").ap()
    out_d = nc.dram_tensor("out", [T, D], bf16, kind="ExternalOutput").ap()

    with tile.TileContext(nc) as tc:
      with tc.tile_pool(name="const", bufs=1) as cpool:
        bo_row = cpool.tile([1, D], f32, name="bo_row")
        bv_row = cpool.tile([1, CPC], f32, name="bv_row")
        bkv_sb = cpool.tile([P, NG], f32, name="bkv_sb")
        bq_sb = cpool.tile([P, NG], f32, name="bq_sb")
        # tiny bias loads go on the (idle) scalar DMA queue: each DMA costs
        # ~0.6us of serial descriptor-issue time, which would delay the
        # weight streams at the head of the sync queue
        nc.scalar.dma_start(out=bo_row[:], in_=bo_d[:].unsqueeze(0))
        nc.scalar.dma_start(out=bv_row[:],
                            in_=bkv_d[CPC:2 * CPC].unsqueeze(0))
        for g in range(NG):
            nc.scalar.dma_start(out=bkv_sb[:, g:g + 1],
                                in_=bkv_d[g * P:(g + 1) * P].unsqueeze(1))
            nc.scalar.dma_start(out=bq_sb[:, g:g + 1],
                                in_=bq_d[g * P:(g + 1) * P].unsqueeze(1))
        bo_bc = cpool.tile([P, D], f32, name="bo_bc")
        bv_bc = cpool.tile([P, CPC], f32, name="bv_bc")
        nc.gpsimd.partition_broadcast(bo_bc[:], bo_row[0:1, :])
        nc.gpsimd.partition_broadcast(bv_bc[:], bv_row[0:1, :])
        # ones row at partition HD for the PE denominator broadcast
        ones64 = cpool.tile([P, HD], bf16, name="ones64")
        nc.vector.memset(ones64[HD:HD + 1, :], 1.0)
        ones0 = cpool.tile([1, P], bf16, name="ones0")
        nc.vector.memset(ones0[:], 1.0)
        bo_row_bf = cpool.tile([1, D], bf16, name="bo_row_bf")
        nc.vector.tensor_copy(bo_row_bf[:], bo_row[:])

        # persistent across A->C
        kT = [cpool.tile([P, T], bf16, name=f"kT{g}") for g in range(NG)]
        # V natural: [128 tok-in-tile, (kt, head, HD+1)], ones col per head
        vnat = cpool.tile([P, NKT * HPC * HD1], bf16, name="vnat")
        nc.vector.memset(
            vnat[:].rearrange("p (k x) -> p k x", x=HD1)[:, :, HD:HD1], 1.0)

        def vn(h, kt):                  # per (head, k-tile) view [P, HD1]
            off = (kt * HPC + h) * HD1
            return vnat[:, off:off + HD1]

        # q parity tiles persist from t=0 so the dead halves can be zeroed
        # while the DVE is otherwise idle (phase A start)
        qTe = [cpool.tile([P, T], bf16, name=f"qTe{g}") for g in range(NG)]
        qTo = [cpool.tile([P, T], bf16, name=f"qTo{g}") for g in range(NG)]
        for g in range(NG):
            nc.vector.memset(qTe[g][HD:P, :], 0.0)
            nc.vector.memset(qTo[g][0:HD, :], 0.0)

        with tc.tile_pool(name="ab", bufs=1) as abpool:
            wkv_sb = [abpool.tile([P, 2 * CPC], bf16, name=f"wkv{d}")
                      for d in range(ND)]
            wq_sb = [abpool.tile([P, CPC], bf16, name=f"wq{d}")
                     for d in range(ND)]

            # ------ Phase A+B merged: per token chunk compute K, V, Q ------
            with tc.tile_pool(name="phA", bufs=1) as apool, \
                 tc.tile_pool(name="phA_ps", bufs=2, space="PSUM") as aps:

                # x chunks are loaded in [P, 2*TC] double-width tiles (one
                # DMA covers two token chunks) on the otherwise-idle gpsimd
                # DMA queue: DMA descriptor issue is serial (~0.6us each) so
                # fewer+bigger transfers off the sync queue matter.
                def load_xpair(tp, which, pool=None, eng=None):
                    src = xeT if which == "e" else xdT
                    eng = eng or nc.gpsimd
                    tcols = slice(tp * 2 * TC, (tp + 1) * 2 * TC)
                    xcs = []
                    for d in range(ND):
                        xc = (pool or apool).tile(
                            [P, 2 * TC], bf16, tag=f"xc{which}{d}",
                            bufs=2, name=f"xc{which}_{tp}_{d}")
                        eng.dma_start(out=xc[:],
                                      in_=src[d * P:(d + 1) * P, tcols])
                        xcs.append(xc)
                    return xcs

                # startup DMA order: tq0/tq1 x_enc + wkv K-half interleaved
                # (first matmul starts after ~2 tiles), then V-half, x_dec,
                # wq; the second pair prefetches inside the loop.
                # first pair split in halves: the tq0 half lands after ~1MB
                # so the first K matmul group starts ~4us earlier
                xcs_pair = []
                for d in range(ND):
                    xc = apool.tile([P, 2 * TC], bf16, tag=f"xce{d}", bufs=2,
                                    name=f"xce_0_{d}")
                    nc.gpsimd.dma_start(out=xc[:, 0:TC],
                                        in_=xeT[d * P:(d + 1) * P, 0:TC])
                    xcs_pair.append(xc)
                    nc.sync.dma_start(out=wkv_sb[d][:, 0:CPC],
                                      in_=wkv_d[d * P:(d + 1) * P, 0:CPC])
                for d in range(ND):
                    nc.gpsimd.dma_start(out=xcs_pair[d][:, TC:2 * TC],
                                        in_=xeT[d * P:(d + 1) * P, TC:2 * TC])
                for d in range(ND):
                    nc.sync.dma_start(out=wkv_sb[d][:, CPC:2 * CPC],
                                      in_=wkv_d[d * P:(d + 1) * P, CPC:2 * CPC])
                # first x_dec pair rides the scalar queue (idle until phase
                # C) so the three early streams split across three queues
                xds_pair = load_xpair(0, "d", eng=nc.scalar)
                for d in range(ND):
                    nc.sync.dma_start(out=wq_sb[d][:],
                                      in_=wq_d[d * P:(d + 1) * P, :])

                for tq in range(NQ):
                    tcols = slice(tq * TC, (tq + 1) * TC)
                    half = (tq % 2) * TC
                    if tq % 2 == 0 and tq > 0:
                        xcs_pair, xds_pair = xcs_next_pair, xds_next_pair
                    if tq % 2 == 0 and tq + 2 < NQ:
                        xcs_next_pair = load_xpair(tq // 2 + 1, "e")
                        # the last x_dec pair outlives phase A/B (its final
                        # Q groups are injected into the attention stream)
                        xds_next_pair = load_xpair(tq // 2 + 1, "d",
                                                   pool=abpool)
                    xcs = [x[:, half:half + TC] for x in xcs_pair]
                    xds = [x[:, half:half + TC] for x in xds_pair]
                    for g in range(NG):          # K groups
                        pkv = aps.tile([P, TC], f32, tag="pk", bufs=3,
                                       name=f"pk_{tq}_{g}")
                        for d in range(ND):
                            nc.tensor.matmul(pkv[:],
                                             wkv_sb[d][:, g * P:(g + 1) * P],
                                             xcs[d][:],
                                             start=(d == 0), stop=(d == ND - 1))
                        nc.vector.tensor_scalar_add(
                            kT[g][:, tcols], pkv[:], bkv_sb[:, g:g + 1])
                    for sub in range(NSUB):      # V natural (x stationary)
                        kt = tq * NSUB + sub
                        pv = aps.tile([P, CPC], f32, tag="pv",
                                      name=f"pv_{tq}_{sub}")
                        for d in range(ND):
                            nc.tensor.matmul(
                                pv[:],
                                xcs[d][:, sub * P:(sub + 1) * P],
                                wkv_sb[d][:, CPC:2 * CPC],
                                start=(d == 0), stop=(d == ND - 1))
                        dst = vnat[:, kt * HPC * HD1:(kt + 1) * HPC * HD1] \
                            .rearrange("p (h x) -> p h x", h=HPC)
                        nc.vector.tensor_add(
                            dst[:, :, 0:HD],
                            pv[:].rearrange("p (h x) -> p h x", h=HPC),
                            bv_bc[:].rearrange("p (h x) -> p h x", h=HPC))
                    if tq == NQ - 1:
                        # defer the last Q groups: they are injected into the
                        # attention stream's first steps so exp starts sooner
                        qinj = {"xds": xds, "tcols": tcols}
                        continue
                    for g in range(NG):          # Q groups (phase B merged)
                        pq = aps.tile([P, TC], f32, tag="pq", bufs=3,
                                      name=f"pq_{tq}_{g}")
                        for d in range(ND):
                            nc.tensor.matmul(
                                pq[:],
                                wq_sb[d][:, g * P:(g + 1) * P],
                                xds[d][:],
                                start=(d == 0), stop=(d == ND - 1))
                        nc.vector.tensor_scalar_add(
                            qTe[g][0:HD, tcols], pq[0:HD, :],
                            bq_sb[0:HD, g:g + 1])
                        nc.vector.tensor_scalar_add(
                            qTo[g][HD:P, tcols], pq[HD:P, :],
                            bq_sb[HD:P, g:g + 1])

            # pool for C/D weights
            with tc.tile_pool(name="late", bufs=1) as lpool:
                wo_sb = [lpool.tile([P, D], bf16, name=f"wo{g}")
                         for g in range(NG)]
                for g in range(NG):
                    nc.sync.dma_start(out=wo_sb[g][:],
                                      in_=wo_d[g * P:(g + 1) * P, :])

                # ----------- Phase C/D: attention + out-projection -----------
                with tc.tile_pool(name="phC", bufs=1) as cp2, \
                     tc.tile_pool(name="phD_ps", bufs=2, space="PSUM") as psD:
                    psS_cm = tc.tile_pool(name="phC_psS", bufs=2, space="PSUM")
                    psO_cm = tc.tile_pool(name="phC_psO", bufs=2, space="PSUM")
                    psS = psS_cm.__enter__()
                    psO = psO_cm.__enter__()

                    ost_map = {}

                    def outproj_half(qc, qt, o, pool=None, pbufs=2,
                                     drain=False):
                        yTs = yTs_by_qc[qc]
                        if o == 0:
                            ost_map[(qc, qt)] = cp2.tile(
                                [P, D], bf16, tag="ost", bufs=4,
                                name=f"ost_{qc}_{qt}")
                        ost = ost_map[(qc, qt)]
                        ocols = slice(o * ON, (o + 1) * ON)
                        pout = (pool or psD).tile([P, ON], f32, tag="pout",
                                                  bufs=pbufs,
                                                  name=f"pout_{qc}_{qt}_{o}")
                        if drain:
                            # bias via ones-row matmul; PSUM->SBUF copy on the
                            # (idle at drain time) ACT engine instead of DVE
                            nc.tensor.matmul(pout[:], ones0[0:1, :],
                                             bo_row_bf[0:1, ocols],
                                             start=True, stop=False,
                                             skip_group_check=True)
                        for g in range(NG):
                            nc.tensor.matmul(
                                pout[:],
                                yTs[g][:, qt * P:(qt + 1) * P],
                                wo_sb[g][:, ocols],
                                start=(g == 0 and not drain),
                                stop=(g == NG - 1),
                                skip_group_check=True)
                        if drain:
                            nc.scalar.activation(ost[:, ocols], pout[:],
                                                 AF.Copy)
                        else:
                            nc.vector.tensor_add(ost[:, ocols], pout[:],
                                                 bo_bc[:, ocols])
                        if o == NON - 1:
                            row0 = (qc * (QC // P) + qt) * P
                            eng = nc.gpsimd if qt % 2 else nc.sync
                            eng.dma_start(out=out_d[row0:row0 + P, :],
                                          in_=ost[:])

                    # per-(qc,g) item state for the deferred-normalize pipeline
                    def make_item(qc, g, yTs):
                        qcols = slice(qc * QC, (qc + 1) * QC)
                        st = {"qc": qc, "g": g, "yTs": yTs, "pss": {},
                              "pts": {}, "po": None}

                        def mm1(kt):
                            ps = psS.tile([P, 2 * QC], f32, tag="ps",
                                          name=f"ps_{qc}_{g}_{kt}")
                            for h2, qT in ((0, qTe), (1, qTo)):
                                nc.tensor.matmul(
                                    ps[:, h2 * QC:(h2 + 1) * QC],
                                    kT[g][:, kt * P:(kt + 1) * P],
                                    qT[g][:, qcols],
                                    skip_group_check=True)
                            st["pss"][kt] = ps

                        def do_exp(kt):
                            pt = cp2.tile([P, 2 * QC], bf16, tag="pt",
                                          bufs=6, name=f"pt_{qc}_{g}_{kt}")
                            nc.scalar.activation(pt[:], st["pss"][kt][:],
                                                 AF.Exp, scale=SCALE)
                            st["pts"][kt] = pt

                        def mm2(kt):
                            pt = st["pts"][kt]
                            for h2 in range(2):
                                nc.tensor.matmul(
                                    st["po"][h2][0:HD1, :],
                                    vn(2 * g + h2, kt),
                                    pt[:, h2 * QC:(h2 + 1) * QC],
                                    start=(kt == 0), stop=(kt == NKT - 1),
                                    skip_group_check=True)

                        st["mm1"], st["exp"], st["mm2"] = mm1, do_exp, mm2
                        return st

                    # normalize pipeline, spread over 4 stream steps so each
                    # hop (PE->DVE->PE->DVE->DVE) has a full step of slack
                    def norm_srow(st):
                        qc, g, po = st["qc"], st["g"], st["po"]
                        st["srow"] = [cp2.tile([P, QC], bf16, tag="srow",
                                               bufs=2,
                                               name=f"srow_{qc}_{g}_{h2}")
                                      for h2 in range(2)]
                        for h2 in range(2):
                            nc.vector.tensor_copy(st["srow"][h2][HD:HD1, :],
                                                  po[h2][HD:HD1, :])

                    def norm_psbc(st):
                        qc, g = st["qc"], st["g"]
                        st["ps_bc"] = [psD.tile([P, QC], f32, tag="pout",
                                                name=f"psbc_{qc}_{g}_{h2}")
                                       for h2 in range(2)]
                        for h2 in range(2):
                            nc.tensor.matmul(st["ps_bc"][h2][0:HD, :],
                                             ones64[HD:HD + 1, :],
                                             st["srow"][h2][HD:HD1, :],
                                             skip_group_check=True)

                    def norm_recip(st):
                        qc, g = st["qc"], st["g"]
                        st["rbc"] = [cp2.tile([P, QC], f32, tag="rbc", bufs=2,
                                              name=f"rbc_{qc}_{g}_{h2}")
                                     for h2 in range(2)]
                        for h2 in range(2):
                            nc.vector.reciprocal_approx_fast(
                                out=st["rbc"][h2][0:HD, :],
                                in_=st["ps_bc"][h2][0:HD, :])

                    def norm_muls(st):
                        qc, g, po, yTs = st["qc"], st["g"], st["po"], st["yTs"]
                        rbc = st["rbc"]
                        nc.vector.tensor_mul(yTs[g][0:HD, :],
                                             po[0][0:HD, :], rbc[0][0:HD, :])
                        stg = cp2.tile([P, QC], bf16, tag="stg",
                                       bufs=2, name=f"stg_{qc}_{g}")
                        nc.vector.tensor_mul(stg[0:HD, :],
                                             po[1][0:HD, :], rbc[1][0:HD, :])
                        nc.sync.dma_start(out=yTs[g][HD:P, :],
                                          in_=stg[0:HD, :])

                    yTs_by_qc = {}
                    for qc in range(NQC):
                        yTs_by_qc[qc] = [
                            cp2.tile([P, QC], bf16, tag=f"yT{g}", bufs=2,
                                     name=f"yT_{qc}_{g}")
                            for g in range(NG)]

                    # global interleaved stream over (item, kt): mm1 at step
                    # n, exp at n-1, mm2 at n-3; item boundaries overlap so
                    # neither PE nor ACT ever drains
                    NI = NQC * NG                  # 16 items
                    N = NI * NKT                   # 256 steps
                    MM2_LAG = 3
                    states = {}

                    def get_state(j):
                        if j not in states:
                            qc, g = divmod(j, NG)
                            states[j] = make_item(qc, g, yTs_by_qc[qc])
                        return states[j]

                    # outproj half-parts for qc-1 spread across qc's items
                    opro = {}
                    for qc in range(1, NQC):
                        for qt in range(QC // P):
                            opro.setdefault(64 * qc + 16 * qt + 12, []) \
                                .append((qc - 1, qt, 0))
                            opro.setdefault(64 * qc + 16 * qt + 14, []) \
                                .append((qc - 1, qt, 1))

                    for n in range(N + MM2_LAG + 4):
                        if n < N:
                            j, kt = divmod(n, NKT)
                            get_state(j)["mm1"](kt)
                        m = n - 1
                        if 0 <= m < N:
                            j, kt = divmod(m, NKT)
                            states[j]["exp"](kt)
                        if n < 4 * NG:
                            # deferred last-chunk Q projection, trickled in
                            # 2 matmuls per step on the psD banks (idle until
                            # the first normalize at step 18)
                            g4, dd = divmod(n, 4)
                            if dd == 0:
                                qinj["ps"] = psD.tile([P, TC], f32,
                                                      tag="pout",
                                                      name=f"pqI_{g4}")
                            for d in (2 * dd, 2 * dd + 1):
                                nc.tensor.matmul(
                                    qinj["ps"][:],
                                    wq_sb[d][:, g4 * P:(g4 + 1) * P],
                                    qinj["xds"][d][:],
                                    start=(d == 0), stop=(d == ND - 1))
                            if dd == 3:
                                tc3 = qinj["tcols"]
                                nc.vector.tensor_scalar_add(
                                    qTe[g4][0:HD, tc3], qinj["ps"][0:HD, :],
                                    bq_sb[0:HD, g4:g4 + 1])
                                nc.vector.tensor_scalar_add(
                                    qTo[g4][HD:P, tc3], qinj["ps"][HD:P, :],
                                    bq_sb[HD:P, g4:g4 + 1])
                        m = n - MM2_LAG
                        if 0 <= m < N:
                            j, kt = divmod(m, NKT)
                            st = states[j]
                            if kt == 0:
                                st["po"] = [psO.tile([P, QC], f32, tag="po",
                                                     name=f"po_{j}_{h2}")
                                            for h2 in range(2)]
                            st["mm2"](kt)
                        for stage, fn in enumerate(
                                (norm_srow, norm_psbc, norm_recip, norm_muls)):
                            b = n - (NKT - 1) - MM2_LAG - stage
                            if b >= 0 and b % NKT == 0 and b // NKT < NI:
                                fn(states[b // NKT])
                        for args in opro.get(n, ()):
                            outproj_half(*args)

                    psO_cm.__exit__(None, None, None)
                    psS_cm.__exit__(None, None, None)
                    # drain: the last q-chunk's out-projection gets the freed
                    # PSUM banks so all its accumulations can fly at once
                    with tc.tile_pool(name="drain_ps", bufs=6,
                                      space="PSUM") as dps:
                        for qt in range(QC // P):
                            for o in range(NON):
                                outproj_half(NQC - 1, qt, o, pool=dps,
                                             pbufs=6, drain=True)

    nc.compile()
    return nc


# ---------------------------------------------------------------------------
# Host side: sharding, run, unshard
# ---------------------------------------------------------------------------

_NC_CACHE = {}


def _get_nc():
    key = "full"
    if key not in _NC_CACHE:
        _NC_CACHE[key] = build_nc()
    return _NC_CACHE[key]


def _group_kv_cols(w_slice, HPC, HD):
    """Reorder kv columns [h-major, (k|v), d] -> K head-major then V."""
    last = w_slice.shape[-1]
    assert last == HPC * 2 * HD
    arr = w_slice.reshape(w_slice.shape[:-1] + (HPC, 2, HD))
    kpart = arr[..., :, 0, :].reshape(w_slice.shape[:-1] + (HPC * HD,))
    vpart = arr[..., :, 1, :].reshape(w_slice.shape[:-1] + (HPC * HD,))
    return np.ascontiguousarray(np.concatenate([kpart, vpart], axis=-1))


def make_in_maps(x_enc, x_dec, Wq, bq, Wkv, bkv, Wo, bo, n_cores=8,
                 HPC=8, HD=64):
    CPC = HPC * HD
    in_maps = []
    xet = [np.ascontiguousarray(x_enc[b].T).astype(NP_BF16)
           for b in range(x_enc.shape[0])]
    xdt = [np.ascontiguousarray(x_dec[b].T).astype(NP_BF16)
           for b in range(x_dec.shape[0])]
    for c in range(n_cores):
        b, hg = c // 2, c % 2
        wkv_slice = Wkv[:, hg * 2 * CPC:(hg + 1) * 2 * CPC]
        bkv_slice = bkv[hg * 2 * CPC:(hg + 1) * 2 * CPC]
        bkv_g = _group_kv_cols(bkv_slice, HPC, HD)
        bo_c = bo if hg == 0 else np.zeros_like(bo)
        in_maps.append({
            "x_enc_t": xet[b],
            "x_dec_t": xdt[b],
            "wq": np.ascontiguousarray(
                Wq[:, hg * CPC:(hg + 1) * CPC]).astype(NP_BF16),
            "wkv_g": _group_kv_cols(wkv_slice, HPC, HD).astype(NP_BF16),
            "wo": np.ascontiguousarray(
                Wo[hg * CPC:(hg + 1) * CPC, :]).astype(NP_BF16),
            "bq": np.ascontiguousarray(bq[hg * CPC:(hg + 1) * CPC]),
            "bkv_g": np.ascontiguousarray(bkv_g),
            "bo": np.ascontiguousarray(bo_c),
        })
    return in_maps


def kernel(x_enc, x_dec, mask, Wq, bq, Wkv, bkv, Wo, bo):
    x_enc = np.asarray(x_enc, dtype=np.float32)
    x_dec = np.asarray(x_dec, dtype=np.float32)
    Wq = np.asarray(Wq, dtype=np.float32)
    bq = np.asarray(bq, dtype=np.float32)
    Wkv = np.asarray(Wkv, dtype=np.float32)
    bkv = np.asarray(bkv, dtype=np.float32)
    Wo = np.asarray(Wo, dtype=np.float32)
    bo = np.asarray(bo, dtype=np.float32)
    mask = np.asarray(mask)
    if mask.any():
        raise ValueError("kernel assumes a zero additive mask (spec fill=zeros)")

    nc = _get_nc()
    in_maps = make_in_maps(x_enc, x_dec, Wq, bq, Wkv, bkv, Wo, bo)
    res = run_bass_kernel_spmd(nc, in_maps, core_ids=list(range(8)))
    outs = [np.asarray(res.results[c]["out"], dtype=np.float32)
            for c in range(8)]
    B = x_enc.shape[0]
    full = np.stack([outs[2 * b] + outs[2 * b + 1] for b in range(B)], axis=0)
    return full


if __name__ == "__main__":
    import time
    t0 = time.time()
    nc = _get_nc()
    print(f"build+compile ok in {time.time() - t0:.1f}s")


# revision 47
# speedup vs baseline: 1.0035x; 1.0035x over previous
"""Multi-head cross-attention on Trainium2, 8-core SPMD (bf16 matmul path).

Problem (hardcoded): B=4, T=2048, D=1024, H=16 heads, head_dim=64.
    kv = x_enc @ Wkv + bkv ; q = x_dec @ Wq + bq
    per head: S = q_h k_h^T / sqrt(64); P = softmax(S + mask); O_h = P v_h
    out = concat_h(O_h) @ Wo + bo

Sharding: data parallel over batch (4 slices x 2 cores each) and tensor
parallel over heads within each pair (8 heads per core).  Each core
computes a partial output  Y_local @ Wo[rows_local]  (+bo on the even
core of the pair); the host unshards by summing each pair's partials
and stacking the 4 batch slices.  Host-side shard prep pre-transposes
the activation matrices, regroups Wkv columns, and converts all matmul
operands to bf16 (fp32 accumulation in PSUM; rel-err budget 2e-2).

The mask input is structurally zero for this problem; softmax(S+0) ==
softmax(S), so the kernel does not load it (checked on the host).

Per-core pipeline (bf16 matmul inputs, fp32 PSUM):
  A+B (merged, per 512-token chunk): K^T = Wkv_K^T x_enc^T (head-pair-
     stacked); V natural directly via x-chunk-stationary matmuls
     (out[tok, vch] = x_sub^T Wv) with the V bias added on the
     PSUM->SBUF copy and a per-head ones column for the softmax
     denominator; Q^T = Wq^T x_dec^T into two parity copies whose
     opposite-parity partitions are zeroed once at t=0.  x tiles stream
     in [128, 1024] double-chunks on the gpsimd DMA queue, weights on
     the sync queue, tiny bias loads on the scalar queue (DMA
     descriptor issue is serial, ~0.6us each, so queue placement and
     transfer sizing dominate the phase ramp).
  C: one software-pipelined stream over all (q-chunk, head-pair) items
     x 16 k-tiles: per step, score matmuls S^T = (K^T kt)^T Q^T_parity
     at n, exp(S^T/8) on ACT (bf16 out) at n-1, O'^T = V_aug^T P^T
     PSUM-accumulation at n-3.  Item boundaries overlap inside the
     stream so neither PE nor ACT ever drains (phase C runs at the ACT
     exp roofline).  Normalization is spread over 4 trailing steps
     (PSUM row copy -> PE ones-matmul denominator broadcast -> fast
     reciprocal -> multiplies) and the out-projection is issued in
     per-128-row halves at fixed steps one q-chunk behind.
  D drain: the last q-chunk's out-projection runs after the stream with
     the freed PSUM banks, bias folded in via a ones-row matmul and the
     PSUM->SBUF copy on the then-idle ACT engine; bf16 stores.
"""

import ml_dtypes
import numpy as np

import concourse.bass as bass
import concourse.mybir as mybir
import concourse.tile as tile
from concourse import bacc
from concourse.bass_utils import run_bass_kernel_spmd

f32 = mybir.dt.float32
bf16 = mybir.dt.bfloat16
AF = mybir.ActivationFunctionType
ALU = mybir.AluOpType

P = 128
NP_BF16 = ml_dtypes.bfloat16


def build_nc(T=2048, D=1024, HPC=8, HD=64, n_cores=8):
    """Build + compile the per-core Bass program. HPC = heads per core."""
    assert HD == 64 and HPC % 2 == 0 and T % 512 == 0 and D % P == 0
    CPC = HPC * HD          # q/out channels per core (512)
    TC = 512                # token chunk (psum free dim), phases A/B
    QC = 512                # q chunk, attention phase
    NQ = T // TC            # token chunks (4)
    ND = D // P             # model-dim chunks (8)
    NG = HPC // 2           # head pairs (4)
    NKT = T // P            # k-token tiles (16)
    NSUB = TC // P          # 128-token subtiles per chunk (4)
    HD1 = HD + 1            # V columns + ones column
    SCALE = float(1.0 / np.sqrt(HD))
    ON = min(512, D)        # out-proj free chunk
    NON = D // ON
    NQC = T // QC

    nc = bacc.Bacc("TRN2", target_bir_lowering=False, debug=False,
                   enable_asserts=False, num_devices=n_cores)

    xeT = nc.dram_tensor("x_enc_t", [D, T], bf16, kind="ExternalInput").ap()
    xdT = nc.dram_tensor("x_dec_t", [D, T], bf16, kind="ExternalInput").ap()
    wq_d = nc.dram_tensor("wq", [D, CPC], bf16, kind="ExternalInput").ap()
    wkv_d = nc.dram_tensor("wkv_g", [D, 2 * CPC], bf16, kind="ExternalInput").ap()
    wo_d = nc.dram_tensor("wo", [CPC, D], bf16, kind="ExternalInput").ap()
    bq_d = nc.dram_tensor("bq", [CPC], f32, kind="ExternalInput").ap()
    bkv_d = nc.dram_tensor("bkv_g", [2 * CPC], f32, kind="ExternalInput").ap()
    bo_d = nc.dram_tensor("bo", [D], f32, kind="ExternalInput").ap()
    out_d = nc.dram_tensor("out", [T, D], bf16, kind="ExternalOutput").ap()

    with tile.TileContext(nc) as tc:
      with tc.tile_pool(name="const", bufs=1) as cpool:
        bo_row = cpool.tile([1, D], f32, name="bo_row")
        bv_row = cpool.tile([1, CPC], f32, name="bv_row")
        bkv_sb = cpool.tile([P, NG], f32, name="bkv_sb")
        bq_sb = cpool.tile([P, NG], f32, name="bq_sb")
        # tiny bias loads go on the (idle) scalar DMA queue: each DMA costs
        # ~0.6us of serial descriptor-issue time, which would delay the
        # weight streams at the head of the sync queue
        nc.scalar.dma_start(out=bo_row[:], in_=bo_d[:].unsqueeze(0))
        nc.scalar.dma_start(out=bv_row[:],
                            in_=bkv_d[CPC:2 * CPC].unsqueeze(0))
        for g in range(NG):
            nc.scalar.dma_start(out=bkv_sb[:, g:g + 1],
                                in_=bkv_d[g * P:(g + 1) * P].unsqueeze(1))
            nc.scalar.dma_start(out=bq_sb[:, g:g + 1],
                                in_=bq_d[g * P:(g + 1) * P].unsqueeze(1))
        bo_bc = cpool.tile([P, D], f32, name="bo_bc")
        bv_bc = cpool.tile([P, CPC], f32, name="bv_bc")
        nc.gpsimd.partition_broadcast(bo_bc[:], bo_row[0:1, :])
        nc.gpsimd.partition_broadcast(bv_bc[:], bv_row[0:1, :])
        # ones row at partition HD for the PE denominator broadcast
        ones64 = cpool.tile([P, HD], bf16, name="ones64")
        nc.vector.memset(ones64[HD:HD + 1, :], 1.0)
        ones0 = cpool.tile([1, P], bf16, name="ones0")
        nc.vector.memset(ones0[:], 1.0)
        bo_row_bf = cpool.tile([1, D], bf16, name="bo_row_bf")
        nc.vector.tensor_copy(bo_row_bf[:], bo_row[:])

        # persistent across A->C
        kT = [cpool.tile([P, T], bf16, name=f"kT{g}") for g in range(NG)]
        # V natural: [128 tok-in-tile, (kt, head, HD+1)], ones col per head
        vnat = cpool.tile([P, NKT * HPC * HD1], bf16, name="vnat")
        nc.vector.memset(
            vnat[:].rearrange("p (k x) -> p k x", x=HD1)[:, :, HD:HD1], 1.0)

        def vn(h, kt):                  # per (head, k-tile) view [P, HD1]
            off = (kt * HPC + h) * HD1
            return vnat[:, off:off + HD1]

        # q parity tiles persist from t=0 so the dead halves can be zeroed
        # while the DVE is otherwise idle (phase A start)
        qTe = [cpool.tile([P, T], bf16, name=f"qTe{g}") for g in range(NG)]
        qTo = [cpool.tile([P, T], bf16, name=f"qTo{g}") for g in range(NG)]
        for g in range(NG):
            nc.vector.memset(qTe[g][HD:P, :], 0.0)
            nc.vector.memset(qTo[g][0:HD, :], 0.0)

        with tc.tile_pool(name="ab", bufs=1) as abpool:
            wkv_sb = [abpool.tile([P, 2 * CPC], bf16, name=f"wkv{d}")
                      for d in range(ND)]
            wq_sb = [abpool.tile([P, CPC], bf16, name=f"wq{d}")
                     for d in range(ND)]

            # ------ Phase A+B merged: per token chunk compute K, V, Q ------
            with tc.tile_pool(name="phA", bufs=1) as apool, \
                 tc.tile_pool(name="phA_ps", bufs=2, space="PSUM") as aps:

                # x chunks are loaded in [P, 2*TC] double-width tiles (one
                # DMA covers two token chunks) on the otherwise-idle gpsimd
                # DMA queue: DMA descriptor issue is serial (~0.6us each) so
                # fewer+bigger transfers off the sync queue matter.
                def load_xpair(tp, which, pool=None, eng=None):
                    src = xeT if which == "e" else xdT
                    eng = eng or nc.gpsimd
                    tcols = slice(tp * 2 * TC, (tp + 1) * 2 * TC)
                    xcs = []
                    for d in range(ND):
                        xc = (pool or apool).tile(
                            [P, 2 * TC], bf16, tag=f"xc{which}{d}",
                            bufs=2, name=f"xc{which}_{tp}_{d}")
                        eng.dma_start(out=xc[:],
                                      in_=src[d * P:(d + 1) * P, tcols])
                        xcs.append(xc)
                    return xcs

                # startup DMA order: tq0/tq1 x_enc + wkv K-half interleaved
                # (first matmul starts after ~2 tiles), then V-half, x_dec,
                # wq; the second pair prefetches inside the loop.
                # first pair split in halves: the tq0 half lands after ~1MB
                # so the first K matmul group starts ~4us earlier
                xcs_pair = []
                for d in range(ND):
                    xc = apool.tile([P, 2 * TC], bf16, tag=f"xce{d}", bufs=2,
                                    name=f"xce_0_{d}")
                    nc.gpsimd.dma_start(out=xc[:, 0:TC],
                                        in_=xeT[d * P:(d + 1) * P, 0:TC])
                    xcs_pair.append(xc)
                    nc.sync.dma_start(out=wkv_sb[d][:, 0:CPC],
                                      in_=wkv_d[d * P:(d + 1) * P, 0:CPC])
                for d in range(ND):
                    nc.gpsimd.dma_start(out=xcs_pair[d][:, TC:2 * TC],
                                        in_=xeT[d * P:(d + 1) * P, TC:2 * TC])
                for d in range(ND):
                    nc.sync.dma_start(out=wkv_sb[d][:, CPC:2 * CPC],
                                      in_=wkv_d[d * P:(d + 1) * P, CPC:2 * CPC])
                xds_pair = load_xpair(0, "d")
                for d in range(ND):
                    nc.sync.dma_start(out=wq_sb[d][:],
                                      in_=wq_d[d * P:(d + 1) * P, :])

                for tq in range(NQ):
                    tcols = slice(tq * TC, (tq + 1) * TC)
                    half = (tq % 2) * TC
                    if tq % 2 == 0 and tq > 0:
                        xcs_pair, xds_pair = xcs_next_pair, xds_next_pair
                    if tq % 2 == 0 and tq + 2 < NQ:
                        xcs_next_pair = load_xpair(tq // 2 + 1, "e")
                        # the last x_dec pair outlives phase A/B (its final
                        # Q groups are injected into the attention stream)
                        xds_next_pair = load_xpair(tq // 2 + 1, "d",
                                                   pool=abpool)
                    xcs = [x[:, half:half + TC] for x in xcs_pair]
                    xds = [x[:, half:half + TC] for x in xds_pair]
                    for g in range(NG):          # K groups
                        pkv = aps.tile([P, TC], f32, tag="pk", bufs=3,
                                       name=f"pk_{tq}_{g}")
                        for d in range(ND):
                            nc.tensor.matmul(pkv[:],
                                             wkv_sb[d][:, g * P:(g + 1) * P],
                                             xcs[d][:],
                                             start=(d == 0), stop=(d == ND - 1))
                        nc.vector.tensor_scalar_add(
                            kT[g][:, tcols], pkv[:], bkv_sb[:, g:g + 1])
                    for sub in range(NSUB):      # V natural (x stationary)
                        kt = tq * NSUB + sub
                        pv = aps.tile([P, CPC], f32, tag="pv",
                                      name=f"pv_{tq}_{sub}")
                        for d in range(ND):
                            nc.tensor.matmul(
                                pv[:],
                                xcs[d][:, sub * P:(sub + 1) * P],
                                wkv_sb[d][:, CPC:2 * CPC],
                                start=(d == 0), stop=(d == ND - 1))
                        dst = vnat[:, kt * HPC * HD1:(kt + 1) * HPC * HD1] \
                            .rearrange("p (h x) -> p h x", h=HPC)
                        nc.vector.tensor_add(
                            dst[:, :, 0:HD],
                            pv[:].rearrange("p (h x) -> p h x", h=HPC),
                            bv_bc[:].rearrange("p (h x) -> p h x", h=HPC))
                    if tq == NQ - 1:
                        # defer the last Q groups: they are injected into the
                        # attention stream's first steps so exp starts sooner
                        qinj = {"xds": xds, "tcols": tcols}
                        continue
                    for g in range(NG):          # Q groups (phase B merged)
                        pq = aps.tile([P, TC], f32, tag="pq", bufs=3,
                                      name=f"pq_{tq}_{g}")
                        for d in range(ND):
                            nc.tensor.matmul(
                                pq[:],
                                wq_sb[d][:, g * P:(g + 1) * P],
                                xds[d][:],
                                start=(d == 0), stop=(d == ND - 1))
                        nc.vector.tensor_scalar_add(
                            qTe[g][0:HD, tcols], pq[0:HD, :],
                            bq_sb[0:HD, g:g + 1])
                        nc.vector.tensor_scalar_add(
                            qTo[g][HD:P, tcols], pq[HD:P, :],
                            bq_sb[HD:P, g:g + 1])

            # pool for C/D weights
            with tc.tile_pool(name="late", bufs=1) as lpool:
                wo_sb = [lpool.tile([P, D], bf16, name=f"wo{g}")
                         for g in range(NG)]
                for g in range(NG):
                    nc.sync.dma_start(out=wo_sb[g][:],
                                      in_=wo_d[g * P:(g + 1) * P, :])

                # ----------- Phase C/D: attention + out-projection -----------
                with tc.tile_pool(name="phC", bufs=1) as cp2, \
                     tc.tile_pool(name="phD_ps", bufs=2, space="PSUM") as psD:
                    psS_cm = tc.tile_pool(name="phC_psS", bufs=2, space="PSUM")
                    psO_cm = tc.tile_pool(name="phC_psO", bufs=2, space="PSUM")
                    psS = psS_cm.__enter__()
                    psO = psO_cm.__enter__()

                    ost_map = {}

                    def outproj_half(qc, qt, o, pool=None, pbufs=2,
                                     drain=False):
                        yTs = yTs_by_qc[qc]
                        if o == 0:
                            ost_map[(qc, qt)] = cp2.tile(
                                [P, D], bf16, tag="ost", bufs=4,
                                name=f"ost_{qc}_{qt}")
                        ost = ost_map[(qc, qt)]
                        ocols = slice(o * ON, (o + 1) * ON)
                        pout = (pool or psD).tile([P, ON], f32, tag="pout",
                                                  bufs=pbufs,
                                                  name=f"pout_{qc}_{qt}_{o}")
                        if drain:
                            # bias via ones-row matmul; PSUM->SBUF copy on the
                            # (idle at drain time) ACT engine instead of DVE
                            nc.tensor.matmul(pout[:], ones0[0:1, :],
                                             bo_row_bf[0:1, ocols],
                                             start=True, stop=False,
                                             skip_group_check=True)
                        for g in range(NG):
                            nc.tensor.matmul(
                                pout[:],
                                yTs[g][:, qt * P:(qt + 1) * P],
                                wo_sb[g][:, ocols],
                                start=(g == 0 and not drain),
                                stop=(g == NG - 1),
                                skip_group_check=True)
                        if drain:
                            nc.scalar.activation(ost[:, ocols], pout[:],
                                                 AF.Copy)
                        else:
                            nc.vector.tensor_add(ost[:, ocols], pout[:],
                                                 bo_bc[:, ocols])
                        if o == NON - 1:
                            row0 = (qc * (QC // P) + qt) * P
                            eng = nc.gpsimd if qt % 2 else nc.sync
                            eng.dma_start(out=out_d[row0:row0 + P, :],
                                          in_=ost[:])

                    # per-(qc,g) item state for the deferred-normalize pipeline
                    def make_item(qc, g, yTs):
                        qcols = slice(qc * QC, (qc + 1) * QC)
                        st = {"qc": qc, "g": g, "yTs": yTs, "pss": {},
                              "pts": {}, "po": None}

                        def mm1(kt):
                            ps = psS.tile([P, 2 * QC], f32, tag="ps",
                                          name=f"ps_{qc}_{g}_{kt}")
                            for h2, qT in ((0, qTe), (1, qTo)):
                                nc.tensor.matmul(
                                    ps[:, h2 * QC:(h2 + 1) * QC],
                                    kT[g][:, kt * P:(kt + 1) * P],
                                    qT[g][:, qcols],
                                    skip_group_check=True)
                            st["pss"][kt] = ps

                        def do_exp(kt):
                            pt = cp2.tile([P, 2 * QC], bf16, tag="pt",
                                          bufs=6, name=f"pt_{qc}_{g}_{kt}")
                            nc.scalar.activation(pt[:], st["pss"][kt][:],
                                                 AF.Exp, scale=SCALE)
                            st["pts"][kt] = pt

                        def mm2(kt):
                            pt = st["pts"][kt]
                            for h2 in range(2):
                                nc.tensor.matmul(
                                    st["po"][h2][0:HD1, :],
                                    vn(2 * g + h2, kt),
                                    pt[:, h2 * QC:(h2 + 1) * QC],
                                    start=(kt == 0), stop=(kt == NKT - 1),
                                    skip_group_check=True)

                        st["mm1"], st["exp"], st["mm2"] = mm1, do_exp, mm2
                        return st

                    # normalize pipeline, spread over 4 stream steps so each
                    # hop (PE->DVE->PE->DVE->DVE) has a full step of slack
                    def norm_srow(st):
                        qc, g, po = st["qc"], st["g"], st["po"]
                        st["srow"] = [cp2.tile([P, QC], bf16, tag="srow",
                                               bufs=2,
                                               name=f"srow_{qc}_{g}_{h2}")
                                      for h2 in range(2)]
                        for h2 in range(2):
                            nc.vector.tensor_copy(st["srow"][h2][HD:HD1, :],
                                                  po[h2][HD:HD1, :])

                    def norm_psbc(st):
                        qc, g = st["qc"], st["g"]
                        st["ps_bc"] = [psD.tile([P, QC], f32, tag="pout",
                                                name=f"psbc_{qc}_{g}_{h2}")
                                       for h2 in range(2)]
                        for h2 in range(2):
                            nc.tensor.matmul(st["ps_bc"][h2][0:HD, :],
                                             ones64[HD:HD + 1, :],
                                             st["srow"][h2][HD:HD1, :],
                                             skip_group_check=True)

                    def norm_recip(st):
                        qc, g = st["qc"], st["g"]
                        st["rbc"] = [cp2.tile([P, QC], f32, tag="rbc", bufs=2,
                                              name=f"rbc_{qc}_{g}_{h2}")
                                     for h2 in range(2)]
                        for h2 in range(2):
                            nc.vector.reciprocal_approx_fast(
                                out=st["rbc"][h2][0:HD, :],
                                in_=st["ps_bc"][h2][0:HD, :])

                    def norm_muls(st):
                        qc, g, po, yTs = st["qc"], st["g"], st["po"], st["yTs"]
                        rbc = st["rbc"]
                        nc.vector.tensor_mul(yTs[g][0:HD, :],
                                             po[0][0:HD, :], rbc[0][0:HD, :])
                        stg = cp2.tile([P, QC], bf16, tag="stg",
                                       bufs=2, name=f"stg_{qc}_{g}")
                        nc.vector.tensor_mul(stg[0:HD, :],
                                             po[1][0:HD, :], rbc[1][0:HD, :])
                        nc.sync.dma_start(out=yTs[g][HD:P, :],
                                          in_=stg[0:HD, :])

                    yTs_by_qc = {}
                    for qc in range(NQC):
                        yTs_by_qc[qc] = [
                            cp2.tile([P, QC], bf16, tag=f"yT{g}", bufs=2,
                                     name=f"yT_{qc}_{g}")
                            for g in range(NG)]

                    # global interleaved stream over (item, kt): mm1 at step
                    # n, exp at n-1, mm2 at n-3; item boundaries overlap so
                    # neither PE nor ACT ever drains
                    NI = NQC * NG                  # 16 items
                    N = NI * NKT                   # 256 steps
                    MM2_LAG = 3
                    states = {}

                    def get_state(j):
                        if j not in states:
                            qc, g = divmod(j, NG)
                            states[j] = make_item(qc, g, yTs_by_qc[qc])
                        return states[j]

                    # outproj half-parts for qc-1 spread across qc's items
                    opro = {}
                    for qc in range(1, NQC):
                        for qt in range(QC // P):
                            opro.setdefault(64 * qc + 16 * qt + 12, []) \
                                .append((qc - 1, qt, 0))
                            opro.setdefault(64 * qc + 16 * qt + 14, []) \
                                .append((qc - 1, qt, 1))

                    for n in range(N + MM2_LAG + 4):
                        if n < N:
                            j, kt = divmod(n, NKT)
                            get_state(j)["mm1"](kt)
                        m = n - 1
                        if 0 <= m < N:
                            j, kt = divmod(m, NKT)
                            states[j]["exp"](kt)
                        if n < 4 * NG:
                            # deferred last-chunk Q projection, trickled in
                            # 2 matmuls per step on the psD banks (idle until
                            # the first normalize at step 18)
                            g4, dd = divmod(n, 4)
                            if dd == 0:
                                qinj["ps"] = psD.tile([P, TC], f32,
                                                      tag="pout",
                                                      name=f"pqI_{g4}")
                            for d in (2 * dd, 2 * dd + 1):
                                nc.tensor.matmul(
                                    qinj["ps"][:],
                                    wq_sb[d][:, g4 * P:(g4 + 1) * P],
                                    qinj["xds"][d][:],
                                    start=(d == 0), stop=(d == ND - 1))
                            if dd == 3:
                                tc3 = qinj["tcols"]
                                nc.vector.tensor_scalar_add(
                                    qTe[g4][0:HD, tc3], qinj["ps"][0:HD, :],
                                    bq_sb[0:HD, g4:g4 + 1])
                                nc.vector.tensor_scalar_add(
                                    qTo[g4][HD:P, tc3], qinj["ps"][HD:P, :],
                                    bq_sb[HD:P, g4:g4 + 1])
                        m = n - MM2_LAG
                        if 0 <= m < N:
                            j, kt = divmod(m, NKT)
                            st = states[j]
                            if kt == 0:
                                st["po"] = [psO.tile([P, QC], f32, tag="po",
                                                     name=f"po_{j}_{h2}")
                                            for h2 in range(2)]
                            st["mm2"](kt)
                        for stage, fn in enumerate(
                                (norm_srow, norm_psbc, norm_recip, norm_muls)):
                            b = n - (NKT - 1) - MM2_LAG - stage
                            if b >= 0 and b % NKT == 0 and b // NKT < NI:
                                fn(states[b // NKT])
                        for args in opro.get(n, ()):
                            outproj_half(*args)

                    psO_cm.__exit__(None, None, None)
                    psS_cm.__exit__(None, None, None)
                    # drain: the last q-chunk's out-projection gets the freed
                    # PSUM banks so all its accumulations can fly at once
                    with tc.tile_pool(name="drain_ps", bufs=6,
                                      space="PSUM") as dps:
                        for qt in range(QC // P):
                            for o in range(NON):
                                outproj_half(NQC - 1, qt, o, pool=dps,
                                             pbufs=6, drain=True)

    nc.compile()
    return nc


# ---------------------------------------------------------------------------
# Host side: sharding, run, unshard
# ---------------------------------------------------------------------------

_NC_CACHE = {}


def _get_nc():
    key = "full"
    if key not in _NC_CACHE:
        _NC_CACHE[key] = build_nc()
    return _NC_CACHE[key]


def _group_kv_cols(w_slice, HPC, HD):
    """Reorder kv columns [h-major, (k|v), d] -> K head-major then V."""
    last = w_slice.shape[-1]
    assert last == HPC * 2 * HD
    arr = w_slice.reshape(w_slice.shape[:-1] + (HPC, 2, HD))
    kpart = arr[..., :, 0, :].reshape(w_slice.shape[:-1] + (HPC * HD,))
    vpart = arr[..., :, 1, :].reshape(w_slice.shape[:-1] + (HPC * HD,))
    return np.ascontiguousarray(np.concatenate([kpart, vpart], axis=-1))


def make_in_maps(x_enc, x_dec, Wq, bq, Wkv, bkv, Wo, bo, n_cores=8,
                 HPC=8, HD=64):
    CPC = HPC * HD
    in_maps = []
    xet = [np.ascontiguousarray(x_enc[b].T).astype(NP_BF16)
           for b in range(x_enc.shape[0])]
    xdt = [np.ascontiguousarray(x_dec[b].T).astype(NP_BF16)
           for b in range(x_dec.shape[0])]
    for c in range(n_cores):
        b, hg = c // 2, c % 2
        wkv_slice = Wkv[:, hg * 2 * CPC:(hg + 1) * 2 * CPC]
        bkv_slice = bkv[hg * 2 * CPC:(hg + 1) * 2 * CPC]
        bkv_g = _group_kv_cols(bkv_slice, HPC, HD)
        bo_c = bo if hg == 0 else np.zeros_like(bo)
        in_maps.append({
            "x_enc_t": xet[b],
            "x_dec_t": xdt[b],
            "wq": np.ascontiguousarray(
                Wq[:, hg * CPC:(hg + 1) * CPC]).astype(NP_BF16),
            "wkv_g": _group_kv_cols(wkv_slice, HPC, HD).astype(NP_BF16),
            "wo": np.ascontiguousarray(
                Wo[hg * CPC:(hg + 1) * CPC, :]).astype(NP_BF16),
            "bq": np.ascontiguousarray(bq[hg * CPC:(hg + 1) * CPC]),
            "bkv_g": np.ascontiguousarray(bkv_g),
            "bo": np.ascontiguousarray(bo_c),
        })
    return in_maps


def kernel(x_enc, x_dec, mask, Wq, bq, Wkv, bkv, Wo, bo):
    x_enc = np.asarray(x_enc, dtype=np.float32)
    x_dec = np.asarray(x_dec, dtype=np.float32)
    Wq = np.asarray(Wq, dtype=np.float32)
    bq = np.asarray(bq, dtype=np.float32)
    Wkv = np.asarray(Wkv, dtype=np.float32)
    bkv = np.asarray(bkv, dtype=np.float32)
    Wo = np.asarray(Wo, dtype=np.float32)
    bo = np.asarray(bo, dtype=np.float32)
    mask = np.asarray(mask)
    if mask.any():
        raise ValueError("kernel assumes a zero additive mask (spec fill=zeros)")

    nc = _get_nc()
    in_maps = make_in_maps(x_enc, x_dec, Wq, bq, Wkv, bkv, Wo, bo)
    res = run_bass_kernel_spmd(nc, in_maps, core_ids=list(range(8)))
    outs = [np.asarray(res.results[c]["out"], dtype=np.float32)
            for c in range(8)]
    B = x_enc.shape[0]
    full = np.stack([outs[2 * b] + outs[2 * b + 1] for b in range(B)], axis=0)
    return full


if __name__ == "__main__":
    import time
    t0 = time.time()
    nc = _get_nc()
    print(f"build+compile ok in {time.time() - t0:.1f}s")
